# revision 43
# baseline (speedup 1.0000x reference)
"""Causal self-attention (dense transformer block) on 8 Trainium2 NeuronCores.

Sharding: tensor-parallel over (batch, head-group). Core c handles batch c//2
and heads (c%2)*8 .. (c%2)*8+8 (16 heads of dim 64, hidden 1024, B=4, T=2048).
Each core computes qkv projection for its head slice, causal attention for its
8 heads, and a partial output projection; the host sums the two head-group
partials per batch.

Device algorithm per core (all matmuls bf16, fp32 PSUM accumulation):
  - Interleaved over 4 supersections of 512 tokens: QKV(sc) -> attention(qc=sc)
    -> out-projection(qc=sc), so ACT-engine exp overlaps QKV/out matmuls.
  - QKV: x.T streamed bf16; q,k channel-major bf16 [128, 4, T]; v token-major
    in per-head-pair blocks: even head [65] = [v|1s], odd head [128] =
    zeros + ones col 32 + v at cols 64.. so the PV matmul lands odd-head
    output at partitions 64..127 and the softmax denominators at partitions
    64 (even) / 32 (odd).
  - Attention per (head, 512-query chunk): S^T[kt,q] = k.T @ q (64-contraction
    via PE quadrant placement), ACT exp (scale 1/8, scores bounded so no
    max-subtraction), causal mask on the 4 diagonal kt-chunks via one DVE
    multiply with a precomputed 0/1 bf16 mask, PV accumulated in PSUM.
  - Normalize: per head pair, reciprocal of the two denominator rows (DVE),
    cast bf16, broadcast to partitions 0..63 / 64..127 with a single ones-
    weight PE matmul, evacuate to SBUF, two DVE multiplies write att bf16.
  - Out projection (bf16) with host-folded bias (out_w @ v_bias + out_b).
"""

import numpy as np
import ml_dtypes

import concourse.mybir as mybir
import concourse.tile as tile
from concourse import bacc
from concourse.bass_utils import run_bass_kernel_spmd

F32 = mybir.dt.float32
BF16 = mybir.dt.bfloat16
AF = mybir.ActivationFunctionType
ALU = mybir.AluOpType

B = 4
T = 2048
CIN = 1024
CQK = 512  # q/k/v channels per core
NKC = 8  # input-channel chunks of 128
NM = 4  # q (and k) output M-chunks of 128
SC = 512  # supersection: tokens per QKV chunk == query chunk
NSC = T // SC
NKT = 16  # key 128-token chunks
QC = 512
NH = 8  # local heads
NP = 4  # local head pairs
D = 64
SCALE = 0.125  # 1/sqrt(64)

_CACHED_NC = None


def _build_nc(
    timing_reps=1,
    debug_dump=False,
    sequential=True,
    oproj_filler=False,
    fine=True,
):
    nc = bacc.Bacc("TRN2", target_bir_lowering=False, debug=False, num_devices=8)

    xt_d = nc.dram_tensor("xt", [CIN, T], BF16, kind="ExternalInput")
    wqkv_d = nc.dram_tensor("wqkv", [CIN, 3 * CQK], BF16, kind="ExternalInput")
    bq_d = nc.dram_tensor("bq", [128, NM], F32, kind="ExternalInput")
    bk_d = nc.dram_tensor("bk", [128, NM], F32, kind="ExternalInput")
    wo_d = nc.dram_tensor("wo", [CQK, CIN], BF16, kind="ExternalInput")
    bo_d = nc.dram_tensor("bo", [128, 8], F32, kind="ExternalInput")
    cm_d = nc.dram_tensor("cmask", [128, 4 * QC], BF16, kind="ExternalInput")
    out_d = nc.dram_tensor("out", [CIN, T], F32, kind="ExternalOutput")
    if debug_dump:
        dbg_qk_d = nc.dram_tensor("dbg_qk", [128, 2 * NM * T], BF16, kind="ExternalOutput")
        dbg_v_d = nc.dram_tensor("dbg_v", [128, NKT * NP * 193], BF16, kind="ExternalOutput")
        dbg_att_d = nc.dram_tensor("dbg_att", [128, 4 * T], BF16, kind="ExternalOutput")
        dbg_pair_d = nc.dram_tensor("dbg_pair", [128, 4 * QC], F32, kind="ExternalOutput")
        dbg_xt_d = nc.dram_tensor("dbg_xt", [128, NKC * T], BF16, kind="ExternalOutput")

    xt_r = xt_d.rearrange("(kc p) t -> p kc t", p=128)
    wqkv_r = wqkv_d.rearrange("(kc p) m -> p kc m", p=128)
    wo_r = wo_d.rearrange("(ac p) m -> p ac m", p=128)
    out_r = out_d.rearrange("(oc p) t -> p oc t", p=128)

    with tile.TileContext(nc) as tc:
        with (
            tc.tile_pool(name="const", bufs=1) as constp,
            tc.tile_pool(name="xt", bufs=2) as xtp,
            tc.tile_pool(name="qkv", bufs=1) as qkvp,
            tc.tile_pool(name="pt", bufs=2) as ptp,
            tc.tile_pool(name="att", bufs=1) as attp,
            tc.tile_pool(name="small", bufs=2) as smallp,
            tc.tile_pool(name="oevac", bufs=3) as oevacp,
            tc.tile_pool(name="psum", bufs=1, space="PSUM") as psp,
        ):
            wqkv_t = constp.tile([128, NKC, 3 * CQK], BF16)
            nc.sync.dma_start(wqkv_t[:], wqkv_r[:])
            wo_t = constp.tile([128, 4, CIN], BF16)
            nc.sync.dma_start(wo_t[:], wo_r[:])
            bq_t = constp.tile([128, NM], F32)
            nc.sync.dma_start(bq_t[:], bq_d[:])
            bk_t = constp.tile([128, NM], F32)
            nc.sync.dma_start(bk_t[:], bk_d[:])
            bo_t = constp.tile([128, 8], F32)
            nc.sync.dma_start(bo_t[:], bo_d[:])
            cm_t = constp.tile([128, 4, QC], BF16)
            nc.sync.dma_start(
                cm_t[:], cm_d[:].rearrange("p (j q) -> p j q", q=QC)
            )
            # ones-weights for the denominator broadcast matmuls (base
            # partition must be 0/32/64): even-head recip lives at partition
            # 64, odd-head recip at partition 32.
            wbc_t = constp.tile([128, 128], BF16)
            nc.vector.memset(wbc_t[:], 0.0)
            nc.vector.memset(wbc_t[64:65, 0:64], 1.0)
            nc.vector.memset(wbc_t[32:33, 0:64], 1.0)

            q_t = qkvp.tile([128, NM, T], BF16)
            k_t = qkvp.tile([128, NM, T], BF16)
            # even-head PV weights: [v_h (64) | ones]
            ve_t = qkvp.tile([128, NKT, NP, D + 1], BF16)
            nc.vector.memset(ve_t[:, :, :, D : D + 1], 1.0)
            # odd-head PV weights: zeros except ones at col 32 (denominator
            # -> partition 32) and v_h at cols 64..127 (-> partitions 64..127)
            vo_t = qkvp.tile([128, NKT, NP, 128], BF16)
            nc.vector.memset(vo_t[:, :, :, 0:64], 0.0)
            nc.vector.memset(vo_t[:, :, :, 32:33], 1.0)
            att_t = attp.tile([128, 4, T], BF16)

            def emit_xt_load(sc):
                tsl = slice(sc * SC, (sc + 1) * SC)
                xt_t = xtp.tile([128, NKC, SC], BF16, tag="xt", name="xt_t", bufs=4)
                nc.sync.dma_start(xt_t[:], xt_r[:, :, tsl])
                if debug_dump:
                    nc.sync.dma_start(
                        dbg_xt_d[:].rearrange("p (kc t) -> p kc t", t=T)[:, :, tsl],
                        xt_t[:],
                    )
                return xt_t

            def emit_qkv_m(sc, m, xt_t, tag="s"):
                tsl = slice(sc * SC, (sc + 1) * SC)
                if tag == "s":
                    ps2 = psp.tile([128, 2, QC], F32, tag=tag, bufs=2, name="ps2")
                    ps = ps2[:, 0, :]
                else:
                    ps = psp.tile([128, QC], F32, tag=tag, bufs=2, name="psm")[:]
                for kc in range(NKC):
                    nc.tensor.matmul(
                        ps,
                        wqkv_t[:, kc, m * 128 : (m + 1) * 128],
                        xt_t[:, kc, :],
                        start=(kc == 0),
                        stop=(kc == NKC - 1),
                    )
                dst = q_t if m < NM else k_t
                bias = bq_t if m < NM else bk_t
                nc.vector.tensor_scalar_add(
                    dst[:, m % NM, tsl],
                    ps,
                    bias[:, m % NM : m % NM + 1],
                )

            def emit_qkv_v(sc, ts, xt_t, tag="s"):
                kt = 4 * sc + ts
                if tag == "s":
                    ps2 = psp.tile([128, 2, QC], F32, tag=tag, bufs=2, name="ps2")
                    ps = ps2[:, 0, :]
                else:
                    ps = psp.tile([128, QC], F32, tag=tag, bufs=2, name="psv")[:]
                for kc in range(NKC):
                    nc.tensor.matmul(
                        ps,
                        xt_t[:, kc, ts * 128 : (ts + 1) * 128],
                        wqkv_t[:, kc, 2 * CQK : 3 * CQK],
                        start=(kc == 0),
                        stop=(kc == NKC - 1),
                    )
                psv = ps.rearrange("p (hc eo d) -> p hc eo d", eo=2, d=D)
                nc.vector.tensor_copy(ve_t[:, kt, :, 0:D], psv[:, :, 0, :])
                nc.vector.tensor_copy(vo_t[:, kt, :, 64:128], psv[:, :, 1, :])

            def emit_qkv(sc, xt_t):
                for m in range(2 * NM):
                    emit_qkv_m(sc, m, xt_t)
                for ts in range(4):
                    emit_qkv_v(sc, ts, xt_t)

            def emit_oproj_oc(sc, oc):
                tsl = slice(sc * SC, (sc + 1) * SC)
                po2 = psp.tile([128, 2, QC], F32, tag="s", bufs=2, name="po2")
                po = po2[:, 0, :]
                for ac in range(4):
                    nc.tensor.matmul(
                        po,
                        wo_t[:, ac, oc * 128 : (oc + 1) * 128],
                        att_t[:, ac, tsl],
                        start=(ac == 0),
                        stop=(ac == 3),
                    )
                ot = oevacp.tile([128, QC], F32, tag="oevac", name="ot")
                nc.vector.tensor_scalar_add(ot[:], po, bo_t[:, oc : oc + 1])
                nc.sync.dma_start(out_r[:, oc, tsl], ot[:])

            def emit_attn_out(
                sc,
                fillers=None,
                inline_oproj=True,
                s_fillers=None,
                defer_norm=True,
            ):
                tsl = slice(sc * SC, (sc + 1) * SC)
                nktc = 4 * (sc + 1)
                pending_norm = [None]
                if True:
                    for hc in range(NP):  # noqa: E999 placeholder
                        pts = []
                        for eo in range(2):
                            pb = 64 * eo
                            qs = q_t[pb : pb + 64, hc, tsl]
                            pt = ptp.tile([128, NKT, QC], BF16, tag="pt")
                            for gi, g in enumerate(range(0, nktc, 2)):
                                sp = psp.tile([128, 2, QC], F32, tag="s", bufs=2)
                                for j2 in range(2):
                                    ktc = g + j2
                                    nc.tensor.matmul(
                                        sp[:, j2, :],
                                        k_t[pb : pb + 64, hc, ktc * 128 : (ktc + 1) * 128],
                                        qs,
                                        start=True,
                                        stop=True,
                                    )
                                nc.scalar.activation(
                                    pt[:, g : g + 2, :], sp[:], AF.Exp, scale=SCALE
                                )
                                # causal mask: diagonal groups are the last
                                # two; mask each right after its exp so PV
                                # never waits on one big late mask
                                if g >= 4 * sc:
                                    j = g - 4 * sc
                                    nc.vector.tensor_tensor(
                                        pt[:, g : g + 2, :],
                                        pt[:, g : g + 2, :],
                                        cm_t[:, j : j + 2, :],
                                        ALU.mult,
                                    )
                                if s_fillers and gi % 2 == 1:
                                    s_fillers.pop(0)()
                            pts.append(pt)
                        # previous pair's normalize runs here (DVE) while this
                        # pair's exps (ACT) and PVs (PE) proceed
                        if pending_norm[0] is not None:
                            pending_norm[0]()
                            pending_norm[0] = None
                        ops = []
                        for eo in range(2):
                            vsrc = ve_t if eo == 0 else vo_t
                            ncol = D + 1 if eo == 0 else 128
                            op = psp.tile(
                                [128, QC], F32, tag=f"op{eo}", bufs=2
                            )
                            for ktc in range(nktc):
                                nc.tensor.matmul(
                                    op[0:ncol, :],
                                    vsrc[:, ktc, hc, 0:ncol],
                                    pts[eo][:, ktc, :],
                                    start=(ktc == 0),
                                    stop=(ktc == nktc - 1),
                                )
                            ops.append(op)
                        # normalize both heads of the pair: broadcast the raw
                        # denominators with a ones-weight matmul, evacuate to
                        # SBUF f32, then one full-width in-place reciprocal
                        # (reciprocal_approx_fast no-ops on HW at any nonzero
                        # partition offset). Deferred to the next pair so the
                        # DVE chain overlaps its S/exp phase.
                        def _normalize(ops=ops, hc=hc, last=(sc == NSC - 1 and hc == NP - 1)):
                            if debug_dump and last:
                                dbgp = constp.tile([128, 4, QC], F32, name="dbgp")
                                nc.vector.memset(dbgp[:], 0.0)
                                nc.vector.tensor_copy(dbgp[0:65, 0, :], ops[0][0:65, :])
                                nc.vector.tensor_copy(dbgp[:, 1, :], ops[1][:])
                            den_b = smallp.tile([128, QC], BF16, tag="den_b")
                            nc.vector.tensor_copy(den_b[64:65, :], ops[0][64:65, :])
                            nc.vector.tensor_copy(den_b[32:33, :], ops[1][32:33, :])
                            bcp = psp.tile([128, 2, QC], F32, tag="s", bufs=2, name="bcp")
                            nc.tensor.matmul(
                                bcp[0:64, 0, :],
                                wbc_t[64:65, 0:64],
                                den_b[64:65, :],
                                start=True,
                                stop=True,
                            )
                            nc.tensor.matmul(
                                bcp[64:128, 0, :],
                                wbc_t[32:33, 0:64],
                                den_b[32:33, :],
                                start=True,
                                stop=True,
                            )
                            bcs = smallp.tile([128, QC], F32, tag="bcs")
                            nc.vector.tensor_copy(bcs[:], bcp[:, 0, :])
                            nc.vector.reciprocal_approx_fast(out=bcs[:], in_=bcs[:])
                            if debug_dump and last:
                                nc.vector.tensor_copy(dbgp[:, 3, :], bcs[:])
                                nc.sync.dma_start(
                                    dbg_pair_d[:].rearrange("p (j q) -> p j q", q=QC),
                                    dbgp[:],
                                )
                            nc.vector.tensor_tensor(
                                att_t[0:64, hc, tsl],
                                ops[0][0:64, :],
                                bcs[0:64, :],
                                ALU.mult,
                            )
                            nc.vector.tensor_tensor(
                                att_t[64:128, hc, tsl],
                                ops[1][64:128, :],
                                bcs[64:128, :],
                                ALU.mult,
                            )

                        if defer_norm:
                            pending_norm[0] = _normalize
                        else:
                            _normalize()
                        if fillers:
                            for _ in range(2):
                                if fillers:
                                    fillers.pop(0)()

                    if pending_norm[0] is not None:
                        pending_norm[0]()
                        pending_norm[0] = None
                    if fillers:
                        while fillers:
                            fillers.pop(0)()
                    if inline_oproj:
                        for oc in range(8):
                            emit_oproj_oc(sc, oc)

            def fine_rep(xts, prev_oproj):
                # QKV(0) interleaved with last chunk's out-proj of the
                # previous rep; attention(sc) absorbs QKV(sc+1) between
                # S-groups (op0/op1 psum rings) and out-proj(sc-1) between
                # pairs (s ring)
                qkv0 = [
                    (lambda m: lambda: emit_qkv_m(0, m, xts[0], tag="op0"))(m)
                    for m in range(2 * NM)
                ] + [
                    (lambda ts: lambda: emit_qkv_v(0, ts, xts[0], tag="op1"))(ts)
                    for ts in range(4)
                ]
                for g in qkv0:
                    g()
                    if prev_oproj:
                        prev_oproj.pop(0)()
                while prev_oproj:
                    prev_oproj.pop(0)()
                for sc in range(NSC):
                    s_fillers = (
                        [
                            (lambda m: lambda: emit_qkv_m(sc + 1, m, xts[sc + 1], tag="op0"))(m)
                            for m in range(2 * NM)
                        ]
                        + [
                            (lambda ts: lambda: emit_qkv_v(sc + 1, ts, xts[sc + 1], tag="op1"))(ts)
                            for ts in range(4)
                        ]
                        if sc < NSC - 1
                        else []
                    )
                    oproj_prev = (
                        [
                            (lambda o: lambda: emit_oproj_oc(sc - 1, o))(oc)
                            for oc in range(8)
                        ]
                        if sc > 0
                        else []
                    )
                    emit_attn_out(
                        sc,
                        fillers=oproj_prev,
                        inline_oproj=False,
                        s_fillers=s_fillers,
                        defer_norm=False,
                    )
                    # any qkv groups not absorbed inside the S loops
                    while s_fillers:
                        s_fillers.pop(0)()
                return [
                    (lambda o: lambda: emit_oproj_oc(NSC - 1, o))(oc)
                    for oc in range(8)
                ]

            if fine:
                prev_oproj = []
                for _rep in range(timing_reps):
                    xts = [emit_xt_load(sc) for sc in range(NSC)]
                    prev_oproj = fine_rep(xts, prev_oproj)
                while prev_oproj:
                    prev_oproj.pop(0)()

            for _rep in range(0 if fine else timing_reps):
                xts = [emit_xt_load(sc) for sc in range(NSC)]
                if sequential:
                    for sc in range(NSC):
                        emit_qkv(sc, xts[sc])
                    if oproj_filler:
                        # out-proj of chunk sc-1 runs as PE filler between
                        # the ACT-bound attention pairs of chunk sc
                        for sc in range(NSC):
                            fillers = (
                                [
                                    (lambda s, o: lambda: emit_oproj_oc(s, o))(
                                        sc - 1, oc
                                    )
                                    for oc in range(8)
                                ]
                                if sc > 0
                                else []
                            )
                            emit_attn_out(sc, fillers=fillers, inline_oproj=False)
                        for oc in range(8):
                            emit_oproj_oc(NSC - 1, oc)
                    else:
                        for sc in range(NSC):
                            emit_attn_out(sc)
                else:
                    for sc in range(NSC):
                        emit_qkv(sc, xts[sc])
                        emit_attn_out(sc)

            if debug_dump:
                qk_r = dbg_qk_d.rearrange("p (i m t) -> p i m t", i=2, t=T)
                nc.sync.dma_start(qk_r[:, 0, :, :], q_t[:])
                nc.sync.dma_start(qk_r[:, 1, :, :], k_t[:])
                v_r = dbg_v_d.rearrange("p (kt np c) -> p kt np c", kt=NKT, c=193)
                nc.sync.dma_start(v_r[:, :, :, 0:65], ve_t[:])
                nc.sync.dma_start(v_r[:, :, :, 65:193], vo_t[:])
                nc.sync.dma_start(
                    dbg_att_d[:].rearrange("p (a t) -> p a t", t=T), att_t[:]
                )

    nc.compile()
    return nc


def _host_prep(data, qkv_w, qkv_b, out_w, out_b):
    # causal mask for the 4 diagonal kt-chunks of a 512-query block:
    # key offset 128*j + p visible to query q iff 128*j + p <= q
    p = np.arange(128)[:, None, None]
    j = np.arange(4)[None, :, None]
    q = np.arange(QC)[None, None, :]
    cmask = (128 * j + p <= q).astype(ml_dtypes.bfloat16).reshape(128, 4 * QC)
    cmask = np.ascontiguousarray(cmask)

    in_maps = []
    for c in range(8):
        b = c // 2
        hg = c % 2
        sl = slice(512 * hg, 512 * hg + 512)
        wq = qkv_w[0:1024][sl]
        wk = qkv_w[1024:2048][sl]
        wv = qkv_w[2048:3072][sl]
        wqkv = np.ascontiguousarray(
            np.concatenate([wq, wk, wv], axis=0).T.astype(ml_dtypes.bfloat16)
        )
        bq = np.ascontiguousarray(
            qkv_b[0:1024][sl].reshape(4, 128).T, dtype=np.float32
        )
        bk = np.ascontiguousarray(
            qkv_b[1024:2048][sl].reshape(4, 128).T, dtype=np.float32
        )
        bv = qkv_b[2048:3072][sl]
        wo = np.ascontiguousarray(out_w[:, sl].T.astype(ml_dtypes.bfloat16))
        bo_full = out_w[:, sl].astype(np.float64) @ bv.astype(np.float64)
        if hg == 0:
            bo_full = bo_full + out_b.astype(np.float64)
        bo = np.ascontiguousarray(bo_full.astype(np.float32).reshape(8, 128).T)
        xt = np.ascontiguousarray(data[b].T.astype(ml_dtypes.bfloat16))
        in_maps.append(
            {
                "xt": xt,
                "wqkv": wqkv,
                "bq": bq,
                "bk": bk,
                "wo": wo,
                "bo": bo,
                "cmask": cmask,
            }
        )
    return in_maps


def _host_gather(results):
    outs = []
    for b in range(B):
        acc = results[2 * b]["out"].astype(np.float32) + results[2 * b + 1][
            "out"
        ].astype(np.float32)
        outs.append(acc.T)
    return np.stack(outs, axis=0)


def kernel(data, qkv_w, qkv_b, out_w, out_b):
    global _CACHED_NC
    data = np.asarray(data, dtype=np.float32)
    qkv_w = np.asarray(qkv_w, dtype=np.float32)
    qkv_b = np.asarray(qkv_b, dtype=np.float32)
    out_w = np.asarray(out_w, dtype=np.float32)
    out_b = np.asarray(out_b, dtype=np.float32)

    if _CACHED_NC is None:
        _CACHED_NC = _build_nc()
    in_maps = _host_prep(data, qkv_w, qkv_b, out_w, out_b)
    res = run_bass_kernel_spmd(_CACHED_NC, in_maps, core_ids=list(range(8)))
    return _host_gather(res.results)


# revision 44
# speedup vs baseline: 1.3351x; 1.3351x over previous
"""Causal self-attention (dense transformer block) on 8 Trainium2 NeuronCores.

Sharding: tensor-parallel over (batch, head-group). Core c handles batch c//2
and heads (c%2)*8 .. (c%2)*8+8 (16 heads of dim 64, hidden 1024, B=4, T=2048).
Each core computes qkv projection for its head slice, causal attention for its
8 heads, and a partial output projection; the host sums the two head-group
partials per batch.

Device algorithm per core (all matmuls bf16, fp32 PSUM accumulation):
  - Fine-grained software pipeline over 4 supersections of 512 tokens: all 4
    x.T tiles prefetched up front; QKV(sc+1) matmul groups run as PE fillers
    between attention(sc)'s exp-paced S-groups (on the op0/op1 PSUM rings),
    and out-projection(sc-1) groups fill between head pairs (s ring), so the
    PE stays busy through the ACT-bound attention stretches (~88% busy).
  - QKV: q,k channel-major bf16 [128, 4, T]; v token-major in per-head-pair
    blocks: even head [65] = [v|1s], odd head [128] = zeros + ones col 32 +
    v at cols 64.. so the PV matmul lands odd-head output at partitions
    64..127 directly (no DMA shift) and the softmax denominators at
    partitions 64 (even) / 32 (odd).
  - Attention per (head, 512-query chunk): S^T[kt,q] = k.T @ q (64-contraction
    via PE quadrant placement), ACT exp (scale 1/8, scores bounded so no
    max-subtraction), causal mask per diagonal exp-group via DVE multiply
    with a precomputed 0/1 bf16 mask, PV accumulated in PSUM.
  - Normalize: per head pair, the two raw denominator rows are cast bf16 and
    broadcast to partitions 0..63 / 64..127 with two ones-weight PE matmuls,
    evacuated to SBUF f32, one full-width in-place reciprocal_approx_fast
    (the op silently no-ops on HW at any nonzero partition offset), then two
    DVE multiplies write att bf16. All-f32 tensor_tensor operands: mixing a
    bf16 operand with an f32 PSUM operand miscomputes on HW.
  - Out projection (bf16) with host-folded bias (out_w @ v_bias + out_b).
"""

import numpy as np
import ml_dtypes

import concourse.mybir as mybir
import concourse.tile as tile
from concourse import bacc
from concourse.bass_utils import run_bass_kernel_spmd

F32 = mybir.dt.float32
BF16 = mybir.dt.bfloat16
AF = mybir.ActivationFunctionType
ALU = mybir.AluOpType

B = 4
T = 2048
CIN = 1024
CQK = 512  # q/k/v channels per core
NKC = 8  # input-channel chunks of 128
NM = 4  # q (and k) output M-chunks of 128
SC = 512  # supersection: tokens per QKV chunk == query chunk
NSC = T // SC
NKT = 16  # key 128-token chunks
QC = 512
NH = 8  # local heads
NP = 4  # local head pairs
D = 64
SCALE = 0.125  # 1/sqrt(64)

_CACHED_NC = None


def _build_nc(
    timing_reps=1,
    debug_dump=False,
    sequential=True,
    oproj_filler=False,
    fine=True,
):
    nc = bacc.Bacc("TRN2", target_bir_lowering=False, debug=False, num_devices=8)

    xt_d = nc.dram_tensor("xt", [CIN, T], BF16, kind="ExternalInput")
    wqkv_d = nc.dram_tensor("wqkv", [CIN, 3 * CQK], BF16, kind="ExternalInput")
    bq_d = nc.dram_tensor("bq", [128, NM], F32, kind="ExternalInput")
    bk_d = nc.dram_tensor("bk", [128, NM], F32, kind="ExternalInput")
    wo_d = nc.dram_tensor("wo", [CQK, CIN], BF16, kind="ExternalInput")
    bo_d = nc.dram_tensor("bo", [128, 8], F32, kind="ExternalInput")
    cm_d = nc.dram_tensor("cmask", [128, 4 * QC], BF16, kind="ExternalInput")
    out_d = nc.dram_tensor("out", [CIN, T], F32, kind="ExternalOutput")
    if debug_dump:
        dbg_qk_d = nc.dram_tensor("dbg_qk", [128, 2 * NM * T], BF16, kind="ExternalOutput")
        dbg_v_d = nc.dram_tensor("dbg_v", [128, NKT * NP * 193], BF16, kind="ExternalOutput")
        dbg_att_d = nc.dram_tensor("dbg_att", [128, 4 * T], BF16, kind="ExternalOutput")
        dbg_pair_d = nc.dram_tensor("dbg_pair", [128, 4 * QC], F32, kind="ExternalOutput")
        dbg_xt_d = nc.dram_tensor("dbg_xt", [128, NKC * T], BF16, kind="ExternalOutput")

    xt_r = xt_d.rearrange("(kc p) t -> p kc t", p=128)
    wqkv_r = wqkv_d.rearrange("(kc p) m -> p kc m", p=128)
    wo_r = wo_d.rearrange("(ac p) m -> p ac m", p=128)
    out_r = out_d.rearrange("(oc p) t -> p oc t", p=128)

    with tile.TileContext(nc) as tc:
        with (
            tc.tile_pool(name="const", bufs=1) as constp,
            tc.tile_pool(name="xt", bufs=2) as xtp,
            tc.tile_pool(name="qkv", bufs=1) as qkvp,
            tc.tile_pool(name="pt", bufs=2) as ptp,
            tc.tile_pool(name="att", bufs=1) as attp,
            tc.tile_pool(name="small", bufs=2) as smallp,
            tc.tile_pool(name="oevac", bufs=3) as oevacp,
            tc.tile_pool(name="psum", bufs=1, space="PSUM") as psp,
        ):
            wqkv_t = constp.tile([128, NKC, 3 * CQK], BF16)
            nc.sync.dma_start(wqkv_t[:], wqkv_r[:])
            wo_t = constp.tile([128, 4, CIN], BF16)
            nc.sync.dma_start(wo_t[:], wo_r[:])
            bq_t = constp.tile([128, NM], F32)
            nc.sync.dma_start(bq_t[:], bq_d[:])
            bk_t = constp.tile([128, NM], F32)
            nc.sync.dma_start(bk_t[:], bk_d[:])
            bo_t = constp.tile([128, 8], F32)
            nc.sync.dma_start(bo_t[:], bo_d[:])
            cm_t = constp.tile([128, 4, QC], BF16)
            nc.sync.dma_start(
                cm_t[:], cm_d[:].rearrange("p (j q) -> p j q", q=QC)
            )
            # ones-weights for the denominator broadcast matmuls (base
            # partition must be 0/32/64): even-head recip lives at partition
            # 64, odd-head recip at partition 32.
            wbc_t = constp.tile([128, 128], BF16)
            nc.vector.memset(wbc_t[:], 0.0)
            nc.vector.memset(wbc_t[64:65, 0:64], 1.0)
            nc.vector.memset(wbc_t[32:33, 0:64], 1.0)

            q_t = qkvp.tile([128, NM, T], BF16)
            k_t = qkvp.tile([128, NM, T], BF16)
            # even-head PV weights: [v_h (64) | ones]
            ve_t = qkvp.tile([128, NKT, NP, D + 1], BF16)
            nc.vector.memset(ve_t[:, :, :, D : D + 1], 1.0)
            # odd-head PV weights: zeros except ones at col 32 (denominator
            # -> partition 32) and v_h at cols 64..127 (-> partitions 64..127)
            vo_t = qkvp.tile([128, NKT, NP, 128], BF16)
            nc.vector.memset(vo_t[:, :, :, 0:64], 0.0)
            nc.vector.memset(vo_t[:, :, :, 32:33], 1.0)
            att_t = attp.tile([128, 4, T], BF16)

            def emit_xt_load(sc):
                tsl = slice(sc * SC, (sc + 1) * SC)
                xt_t = xtp.tile([128, NKC, SC], BF16, tag="xt", name="xt_t", bufs=4)
                nc.sync.dma_start(xt_t[:], xt_r[:, :, tsl])
                if debug_dump:
                    nc.sync.dma_start(
                        dbg_xt_d[:].rearrange("p (kc t) -> p kc t", t=T)[:, :, tsl],
                        xt_t[:],
                    )
                return xt_t

            def emit_qkv_m(sc, m, xt_t, tag="s"):
                tsl = slice(sc * SC, (sc + 1) * SC)
                if tag == "s":
                    ps2 = psp.tile([128, 2, QC], F32, tag=tag, bufs=2, name="ps2")
                    ps = ps2[:, 0, :]
                else:
                    ps = psp.tile([128, QC], F32, tag=tag, bufs=2, name="psm")[:]
                for kc in range(NKC):
                    nc.tensor.matmul(
                        ps,
                        wqkv_t[:, kc, m * 128 : (m + 1) * 128],
                        xt_t[:, kc, :],
                        start=(kc == 0),
                        stop=(kc == NKC - 1),
                    )
                dst = q_t if m < NM else k_t
                bias = bq_t if m < NM else bk_t
                nc.vector.tensor_scalar_add(
                    dst[:, m % NM, tsl],
                    ps,
                    bias[:, m % NM : m % NM + 1],
                )

            def emit_qkv_v(sc, ts, xt_t, tag="s"):
                kt = 4 * sc + ts
                if tag == "s":
                    ps2 = psp.tile([128, 2, QC], F32, tag=tag, bufs=2, name="ps2")
                    ps = ps2[:, 0, :]
                else:
                    ps = psp.tile([128, QC], F32, tag=tag, bufs=2, name="psv")[:]
                for kc in range(NKC):
                    nc.tensor.matmul(
                        ps,
                        xt_t[:, kc, ts * 128 : (ts + 1) * 128],
                        wqkv_t[:, kc, 2 * CQK : 3 * CQK],
                        start=(kc == 0),
                        stop=(kc == NKC - 1),
                    )
                psv = ps.rearrange("p (hc eo d) -> p hc eo d", eo=2, d=D)
                nc.vector.tensor_copy(ve_t[:, kt, :, 0:D], psv[:, :, 0, :])
                nc.vector.tensor_copy(vo_t[:, kt, :, 64:128], psv[:, :, 1, :])

            def emit_qkv(sc, xt_t):
                for m in range(2 * NM):
                    emit_qkv_m(sc, m, xt_t)
                for ts in range(4):
                    emit_qkv_v(sc, ts, xt_t)

            def emit_oproj_oc(sc, oc):
                tsl = slice(sc * SC, (sc + 1) * SC)
                po2 = psp.tile([128, 2, QC], F32, tag="s", bufs=2, name="po2")
                po = po2[:, 0, :]
                for ac in range(4):
                    nc.tensor.matmul(
                        po,
                        wo_t[:, ac, oc * 128 : (oc + 1) * 128],
                        att_t[:, ac, tsl],
                        start=(ac == 0),
                        stop=(ac == 3),
                    )
                ot = oevacp.tile([128, QC], F32, tag="oevac", name="ot")
                nc.vector.tensor_scalar_add(ot[:], po, bo_t[:, oc : oc + 1])
                nc.sync.dma_start(out_r[:, oc, tsl], ot[:])

            def emit_attn_out(
                sc,
                fillers=None,
                inline_oproj=True,
                s_fillers=None,
                defer_norm=True,
            ):
                tsl = slice(sc * SC, (sc + 1) * SC)
                nktc = 4 * (sc + 1)
                pending_norm = [None]
                if True:
                    for hc in range(NP):  # noqa: E999 placeholder
                        pts = []
                        for eo in range(2):
                            pb = 64 * eo
                            qs = q_t[pb : pb + 64, hc, tsl]
                            pt = ptp.tile([128, NKT, QC], BF16, tag="pt")
                            for gi, g in enumerate(range(0, nktc, 2)):
                                sp = psp.tile([128, 2, QC], F32, tag="s", bufs=2)
                                for j2 in range(2):
                                    ktc = g + j2
                                    nc.tensor.matmul(
                                        sp[:, j2, :],
                                        k_t[pb : pb + 64, hc, ktc * 128 : (ktc + 1) * 128],
                                        qs,
                                        start=True,
                                        stop=True,
                                    )
                                nc.scalar.activation(
                                    pt[:, g : g + 2, :], sp[:], AF.Exp, scale=SCALE
                                )
                                # causal mask: diagonal groups are the last
                                # two; mask each right after its exp so PV
                                # never waits on one big late mask
                                if g >= 4 * sc:
                                    j = g - 4 * sc
                                    nc.vector.tensor_tensor(
                                        pt[:, g : g + 2, :],
                                        pt[:, g : g + 2, :],
                                        cm_t[:, j : j + 2, :],
                                        ALU.mult,
                                    )
                                if s_fillers and gi % 2 == 1:
                                    s_fillers.pop(0)()
                            pts.append(pt)
                        # previous pair's normalize runs here (DVE) while this
                        # pair's exps (ACT) and PVs (PE) proceed
                        if pending_norm[0] is not None:
                            pending_norm[0]()
                            pending_norm[0] = None
                        ops = []
                        for eo in range(2):
                            vsrc = ve_t if eo == 0 else vo_t
                            ncol = D + 1 if eo == 0 else 128
                            op = psp.tile(
                                [128, QC], F32, tag=f"op{eo}", bufs=2
                            )
                            for ktc in range(nktc):
                                nc.tensor.matmul(
                                    op[0:ncol, :],
                                    vsrc[:, ktc, hc, 0:ncol],
                                    pts[eo][:, ktc, :],
                                    start=(ktc == 0),
                                    stop=(ktc == nktc - 1),
                                )
                            ops.append(op)
                        # normalize both heads of the pair: broadcast the raw
                        # denominators with a ones-weight matmul, evacuate to
                        # SBUF f32, then one full-width in-place reciprocal
                        # (reciprocal_approx_fast no-ops on HW at any nonzero
                        # partition offset). Deferred to the next pair so the
                        # DVE chain overlaps its S/exp phase.
                        def _normalize(ops=ops, hc=hc, last=(sc == NSC - 1 and hc == NP - 1)):
                            if debug_dump and last:
                                dbgp = constp.tile([128, 4, QC], F32, name="dbgp")
                                nc.vector.memset(dbgp[:], 0.0)
                                nc.vector.tensor_copy(dbgp[0:65, 0, :], ops[0][0:65, :])
                                nc.vector.tensor_copy(dbgp[:, 1, :], ops[1][:])
                            den_b = smallp.tile([128, QC], BF16, tag="den_b")
                            nc.vector.tensor_copy(den_b[64:65, :], ops[0][64:65, :])
                            nc.vector.tensor_copy(den_b[32:33, :], ops[1][32:33, :])
                            bcp = psp.tile([128, 2, QC], F32, tag="s", bufs=2, name="bcp")
                            nc.tensor.matmul(
                                bcp[0:64, 0, :],
                                wbc_t[64:65, 0:64],
                                den_b[64:65, :],
                                start=True,
                                stop=True,
                            )
                            nc.tensor.matmul(
                                bcp[64:128, 0, :],
                                wbc_t[32:33, 0:64],
                                den_b[32:33, :],
                                start=True,
                                stop=True,
                            )
                            bcs = smallp.tile([128, QC], F32, tag="bcs")
                            nc.vector.tensor_copy(bcs[:], bcp[:, 0, :])
                            nc.vector.reciprocal_approx_fast(out=bcs[:], in_=bcs[:])
                            if debug_dump and last:
                                nc.vector.tensor_copy(dbgp[:, 3, :], bcs[:])
                                nc.sync.dma_start(
                                    dbg_pair_d[:].rearrange("p (j q) -> p j q", q=QC),
                                    dbgp[:],
                                )
                            nc.vector.tensor_tensor(
                                att_t[0:64, hc, tsl],
                                ops[0][0:64, :],
                                bcs[0:64, :],
                                ALU.mult,
                            )
                            nc.vector.tensor_tensor(
                                att_t[64:128, hc, tsl],
                                ops[1][64:128, :],
                                bcs[64:128, :],
                                ALU.mult,
                            )

                        if defer_norm:
                            pending_norm[0] = _normalize
                        else:
                            _normalize()
                        if fillers:
                            for _ in range(2):
                                if fillers:
                                    fillers.pop(0)()

                    if pending_norm[0] is not None:
                        pending_norm[0]()
                        pending_norm[0] = None
                    if fillers:
                        while fillers:
                            fillers.pop(0)()
                    if inline_oproj:
                        for oc in range(8):
                            emit_oproj_oc(sc, oc)

            def fine_rep(xts, prev_oproj):
                # QKV(0) interleaved with last chunk's out-proj of the
                # previous rep; attention(sc) absorbs QKV(sc+1) between
                # S-groups (op0/op1 psum rings) and out-proj(sc-1) between
                # pairs (s ring)
                qkv0 = [
                    (lambda m: lambda: emit_qkv_m(0, m, xts[0], tag="op0"))(m)
                    for m in range(2 * NM)
                ] + [
                    (lambda ts: lambda: emit_qkv_v(0, ts, xts[0], tag="op1"))(ts)
                    for ts in range(4)
                ]
                for g in qkv0:
                    g()
                    if prev_oproj:
                        prev_oproj.pop(0)()
                while prev_oproj:
                    prev_oproj.pop(0)()
                for sc in range(NSC):
                    s_fillers = (
                        [
                            (lambda m: lambda: emit_qkv_m(sc + 1, m, xts[sc + 1], tag="op0"))(m)
                            for m in range(2 * NM)
                        ]
                        + [
                            (lambda ts: lambda: emit_qkv_v(sc + 1, ts, xts[sc + 1], tag="op1"))(ts)
                            for ts in range(4)
                        ]
                        if sc < NSC - 1
                        else []
                    )
                    oproj_prev = (
                        [
                            (lambda o: lambda: emit_oproj_oc(sc - 1, o))(oc)
                            for oc in range(8)
                        ]
                        if sc > 0
                        else []
                    )
                    emit_attn_out(
                        sc,
                        fillers=oproj_prev,
                        inline_oproj=False,
                        s_fillers=s_fillers,
                        defer_norm=False,
                    )
                    # any qkv groups not absorbed inside the S loops
                    while s_fillers:
                        s_fillers.pop(0)()
                return [
                    (lambda o: lambda: emit_oproj_oc(NSC - 1, o))(oc)
                    for oc in range(8)
                ]

            if fine:
                prev_oproj = []
                for _rep in range(timing_reps):
                    xts = [emit_xt_load(sc) for sc in range(NSC)]
                    prev_oproj = fine_rep(xts, prev_oproj)
                while prev_oproj:
                    prev_oproj.pop(0)()

            for _rep in range(0 if fine else timing_reps):
                xts = [emit_xt_load(sc) for sc in range(NSC)]
                if sequential:
                    for sc in range(NSC):
                        emit_qkv(sc, xts[sc])
                    if oproj_filler:
                        # out-proj of chunk sc-1 runs as PE filler between
                        # the ACT-bound attention pairs of chunk sc
                        for sc in range(NSC):
                            fillers = (
                                [
                                    (lambda s, o: lambda: emit_oproj_oc(s, o))(
                                        sc - 1, oc
                                    )
                                    for oc in range(8)
                                ]
                                if sc > 0
                                else []
                            )
                            emit_attn_out(sc, fillers=fillers, inline_oproj=False)
                        for oc in range(8):
                            emit_oproj_oc(NSC - 1, oc)
                    else:
                        for sc in range(NSC):
                            emit_attn_out(sc)
                else:
                    for sc in range(NSC):
                        emit_qkv(sc, xts[sc])
                        emit_attn_out(sc)

            if debug_dump:
                qk_r = dbg_qk_d.rearrange("p (i m t) -> p i m t", i=2, t=T)
                nc.sync.dma_start(qk_r[:, 0, :, :], q_t[:])
                nc.sync.dma_start(qk_r[:, 1, :, :], k_t[:])
                v_r = dbg_v_d.rearrange("p (kt np c) -> p kt np c", kt=NKT, c=193)
                nc.sync.dma_start(v_r[:, :, :, 0:65], ve_t[:])
                nc.sync.dma_start(v_r[:, :, :, 65:193], vo_t[:])
                nc.sync.dma_start(
                    dbg_att_d[:].rearrange("p (a t) -> p a t", t=T), att_t[:]
                )

    nc.compile()
    return nc


def _host_prep(data, qkv_w, qkv_b, out_w, out_b):
    # causal mask for the 4 diagonal kt-chunks of a 512-query block:
    # key offset 128*j + p visible to query q iff 128*j + p <= q
    p = np.arange(128)[:, None, None]
    j = np.arange(4)[None, :, None]
    q = np.arange(QC)[None, None, :]
    cmask = (128 * j + p <= q).astype(ml_dtypes.bfloat16).reshape(128, 4 * QC)
    cmask = np.ascontiguousarray(cmask)

    in_maps = []
    for c in range(8):
        b = c // 2
        hg = c % 2
        sl = slice(512 * hg, 512 * hg + 512)
        wq = qkv_w[0:1024][sl]
        wk = qkv_w[1024:2048][sl]
        wv = qkv_w[2048:3072][sl]
        wqkv = np.ascontiguousarray(
            np.concatenate([wq, wk, wv], axis=0).T.astype(ml_dtypes.bfloat16)
        )
        bq = np.ascontiguousarray(
            qkv_b[0:1024][sl].reshape(4, 128).T, dtype=np.float32
        )
        bk = np.ascontiguousarray(
            qkv_b[1024:2048][sl].reshape(4, 128).T, dtype=np.float32
        )
        bv = qkv_b[2048:3072][sl]
        wo = np.ascontiguousarray(out_w[:, sl].T.astype(ml_dtypes.bfloat16))
        bo_full = out_w[:, sl].astype(np.float64) @ bv.astype(np.float64)
        if hg == 0:
            bo_full = bo_full + out_b.astype(np.float64)
        bo = np.ascontiguousarray(bo_full.astype(np.float32).reshape(8, 128).T)
        xt = np.ascontiguousarray(data[b].T.astype(ml_dtypes.bfloat16))
        in_maps.append(
            {
                "xt": xt,
                "wqkv": wqkv,
                "bq": bq,
                "bk": bk,
                "wo": wo,
                "bo": bo,
                "cmask": cmask,
            }
        )
    return in_maps


def _host_gather(results):
    outs = []
    for b in range(B):
        acc = results[2 * b]["out"].astype(np.float32) + results[2 * b + 1][
            "out"
        ].astype(np.float32)
        outs.append(acc.T)
    return np.stack(outs, axis=0)


def kernel(data, qkv_w, qkv_b, out_w, out_b):
    global _CACHED_NC
    data = np.asarray(data, dtype=np.float32)
    qkv_w = np.asarray(qkv_w, dtype=np.float32)
    qkv_b = np.asarray(qkv_b, dtype=np.float32)
    out_w = np.asarray(out_w, dtype=np.float32)
    out_b = np.asarray(out_b, dtype=np.float32)

    if _CACHED_NC is None:
        _CACHED_NC = _build_nc()
    in_maps = _host_prep(data, qkv_w, qkv_b, out_w, out_b)
    res = run_bass_kernel_spmd(_CACHED_NC, in_maps, core_ids=list(range(8)))
    return _host_gather(res.results)
